# revision 21
# baseline (speedup 1.0000x reference)
"""CTRNN Trainium2 kernel.

Model (per reference):
    W_eff = (1-1e-5)*I - W_raw^T W_raw           (symmetric, 512x512)
    xp_t  = x_t @ Wi^T + bi + bh                 (input projection)
    r_t   = relu(xp_t + W_eff h_{t-1})
    h_t   = 0.9 h_{t-1} + 0.1 r_t                -> outputs[t] = h_t

Device formulation (no per-step transposes; state kept in
[hidden-on-partition, batch-on-free] orientation):
    p_t = xp_{t+1} + W_eff h_t  (pre-activation for step t+1)
    rho_t = 0.1 * r_t = relu(0.1 * p_{t-1})
        p_t = 0.9 p_{t-1} + xq_t + W_eff rho_t
        h_t = 0.9 h_{t-1} + rho_t
    xq_t = xp_{t+1} - 0.9 xp_t = Wi_ext @ xd_t, computed on device per
    16-step chunk directly into PSUM; the per-step recurrence matmuls
    accumulate on top (start=False).  The bias is folded in through an
    extended constant input channel.

Sharding: data-parallel over batch (16 rows/core), weights replicated.

NOTE on sync: the TRN2 Matmult/LDWEIGHTS encoding supports a single sync
wait.  The kernel is arranged so every PE instruction needs at most one
fresh semaphore: single-tensor DMAs, dummy "observer" matmuls that absorb
one new semaphore each, relu on DVE (so the per-step PE wait on rho also
dominates DVE's older psum reads), and all transpose evacuation copies on
one engine (ACT) absorbed by a dummy transpose.
"""

import sys

if "/opt/trn_rl_repo" not in sys.path:
    sys.path.insert(0, "/opt/trn_rl_repo")

import numpy as np

import concourse.bass as bass
import concourse.mybir as mybir
from concourse.masks import make_identity
from concourse.tile import TileContext

SEQ, BATCH, IN, HID = 1024, 128, 256, 512
NCORES = 8
BC = BATCH // NCORES  # 16 batch rows per core
CHUNK = 16            # steps per xproj chunk (psum-bank resident)
ALPHA = 0.1
DECAY = 1.0 - ALPHA

dt = mybir.dt
AOT = mybir.AluOpType


def _split_multi_waits(nc: bass.Bass) -> None:
    """Walrus' TPB engine-instruction encodings accept a single sync wait.
    Tile sometimes attaches several; move the extras onto injected
    sequencer NOPs just before the instruction (same engine queue, so
    ordering and the semaphore semantics are preserved)."""
    ctr = 0
    for f in nc.m.functions:
        for b in f.blocks:
            insts = b.instructions
            if not any(
                i.sync_info is not None and len(i.sync_info.on_wait) > 1 for i in insts
            ):
                continue
            out = []
            for i in insts:
                si = i.sync_info
                if si is not None and len(si.on_wait) > 1:
                    waits = list(si.on_wait)
                    for w in waits[:-1]:
                        ctr += 1
                        nop = mybir.InstNoOp(name=f"I-wsplit-{ctr}", ins=[], outs=[])
                        nop.engine = i.engine
                        nop.sync_info = mybir.SyncInfo(on_wait=[w], on_update=[])
                        out.append(nop)
                    i.sync_info = mybir.SyncInfo(
                        on_wait=[waits[-1]], on_update=list(si.on_update)
                    )
                out.append(i)
            b.instructions = out


import os

MM_ORDER = os.environ.get("MM_ORDER", "b0_split")
FILLER_POS = os.environ.get("FILLER_POS", "before")
RELU_ENGINE = os.environ.get("RELU_ENGINE", "dve")


def _rec_mm_order():
    """(q, mi, k) emission order for the 16 recurrence matmuls."""
    if MM_ORDER == "m_outer":
        return [(q, mi, k) for q in range(2) for mi in range(2) for k in range(4)]
    if MM_ORDER == "k_phase":
        # all k01 first, then all k23
        seq = [(q, mi, k) for kp in (0, 1) for q in range(2) for mi in range(2)
               for k in (2 * kp, 2 * kp + 1)]
        return seq
    if MM_ORDER == "b0_split":
        # bank0 k01, bank0 k23, bank1 all
        seq = []
        seq += [(0, mi, k) for mi in range(2) for k in (0, 1)]
        seq += [(0, mi, k) for mi in range(2) for k in (2, 3)]
        seq += [(1, mi, k) for mi in range(2) for k in range(4)]
        return seq
    raise ValueError(MM_ORDER)


def build_nc(n_steps: int = SEQ, split_waits: bool = True) -> bass.Bass:
    assert n_steps % CHUNK == 0
    nchunks = n_steps // CHUNK

    nc = bass.Bass(trn_type="TRN2", target_bir_lowering=False, debug=False)

    # xd packed per chunk: [128, nchunks*768]; cols j*768+[0:256)=k0 rows,
    # [256:512)=k1 rows, [512:768) row0 = bias channel
    xd_d = nc.dram_tensor("xd", [128, nchunks * 768], dt.float32r, kind="ExternalInput").ap()
    w_d = nc.dram_tensor("w16", [128, 16 * 128], dt.float16, kind="ExternalInput").ap()
    # wi packed: [128, 1536]: [0:512)=WiT rows 0:128, [512:1024)=rows 128:256,
    # row 0 of [1024:1536) = bias row (WiT row 256)
    wi_d = nc.dram_tensor("wit", [128, 1536], dt.float32r, kind="ExternalInput").ap()
    out_d = nc.dram_tensor("out_bh", [n_steps, BC, HID], dt.float32, kind="ExternalOutput").ap()

    with TileContext(nc) as tc:
        with (
            tc.tile_pool(name="const", bufs=1) as cpool,
            tc.tile_pool(name="sb", bufs=1) as pool,
            tc.tile_pool(name="ps", bufs=1, space="PSUM") as pp,
        ):
            # --- persistent SBUF buffers -------------------------------
            w_sb = cpool.tile([128, 16 * 128], dt.float16)
            wi_sb = cpool.tile([128, 1536], dt.float32r)
            ident = cpool.tile([128, 128], dt.float32)
            pzero = cpool.tile([128, 64], dt.float32)
            hzero = cpool.tile([128, 64], dt.float32)
            xd_t = [pool.tile([128, 768], dt.float32r, name=f"xdt{b}") for b in range(2)]
            rho = [pool.tile([128, 64], dt.float16, name=f"rho{i}") for i in range(2)]
            p_sb = [pool.tile([128, 64], dt.float32, name=f"p{i}") for i in range(2)]
            hist = [pool.tile([128, 512], dt.float32, name=f"hist{i}") for i in range(2)]
            stag = [pool.tile([128, 512], dt.float32, name=f"stag{i}") for i in range(2)]
            psum = [
                [pp.tile([128, 512], dt.float32, name=f"ps{b}_{q}") for q in range(2)]
                for b in range(2)
            ]
            psumT = pp.tile([128, 512], dt.float32, name="psT")

            # --- setup -------------------------------------------------
            nc.sync.dma_start(out=w_sb[:], in_=w_d[:, :])
            nc.sync.dma_start(out=wi_sb[:], in_=wi_d[:, :])
            make_identity(nc, ident[:])
            nc.vector.memset(pzero[:], 0.0)
            nc.gpsimd.memset(hzero[:], 0.0)

            def emit_xd_dma(j: int):
                nc.sync.dma_start(out=xd_t[j % 2][:], in_=xd_d[:, j * 768 : (j + 1) * 768])

            # observer dummies: absorb one fresh DMA semaphore each so real
            # matmuls carry at most one wait
            nc.tensor.matmul(psumT[0:1, 0:1], w_sb[:, 0:1], w_sb[:, 0:1], start=True, stop=True)
            # rows >=1 of the bias block are zero padding -> finite as fp16
            wi_bits = wi_sb[32:33, 1024:1026].bitcast(dt.float16)
            nc.tensor.matmul(psumT[0:1, 0:1], wi_bits[:, 0:1], wi_bits[:, 0:1], start=True, stop=True)

            def xq_mm_closures(j: int):
                b = j % 2
                ops = []
                for q in range(2):
                    for mi in range(2):
                        m = 2 * q + mi
                        for k in range(3):
                            if k < 2:
                                lhsT = wi_sb[:, k * 512 + m * 128 : k * 512 + (m + 1) * 128]
                                rhs = xd_t[b][:, k * 256 : (k + 1) * 256]
                            else:
                                lhsT = wi_sb[0:1, 1024 + m * 128 : 1024 + (m + 1) * 128]
                                rhs = xd_t[b][0:1, 512 : 768]
                            outp = psum[b][q][:, mi * 256 : (mi + 1) * 256]
                            first = mi == 0 and k == 0

                            def op(lhsT=lhsT, rhs=rhs, outp=outp, first=first):
                                # stop=True everywhere: stop is a HW no-op; it
                                # only closes the sim's read-guard so DVE may
                                # read each slot mid-chunk.  Accumulation is
                                # controlled by start/has_written alone.
                                nc.tensor.matmul(
                                    outp, lhsT, rhs, start=first, stop=True,
                                    skip_group_check=not first,
                                )

                            ops.append(op)
                return ops

            # --- prologue: chunks 0 and 1 in flight --------------------
            emit_xd_dma(0)
            for op in xq_mm_closures(0):
                op()
            if nchunks > 1:
                emit_xd_dma(1)
                for op in xq_mm_closures(1):
                    op()

            pending_xq = []
            pending_out = []

            # --- main unrolled recurrence ------------------------------
            for c in range(n_steps):
                j, s = divmod(c, CHUNK)
                if s == 0:
                    if j + 2 < nchunks:
                        emit_xd_dma(j + 2)
                    if j >= 1 and j + 1 < nchunks:
                        pending_xq = xq_mm_closures(j + 1)

                b = j % 2

                def emit_fillers():
                    nonlocal pending_xq, pending_out
                    # one output-path op (PE transpose / ACT copy / DMA) per step
                    for op in pending_out[:2]:
                        op()
                    pending_out = pending_out[2:]
                    if s >= 1:
                        for op in pending_xq[:2]:
                            op()
                        pending_xq = pending_xq[2:]

                if FILLER_POS == "before":
                    emit_fillers()
                # recurrence matmuls: psum[slot s] += W_eff @ rho_{c-1}
                if c > 0:
                    rprev = rho[(c - 1) % 2]
                    for q, mi, k in _rec_mm_order():
                        m = 2 * q + mi
                        nc.tensor.matmul(
                            psum[b][q][:, mi * 256 + s * 16 : mi * 256 + (s + 1) * 16],
                            w_sb[:, (k * 4 + m) * 128 : (k * 4 + m + 1) * 128],
                            rprev[:, k * 16 : (k + 1) * 16],
                            start=False,
                            stop=True,
                            skip_group_check=True,
                        )
                if FILLER_POS == "after":
                    emit_fillers()

                # p_t = 0.9 p_{t-1} + psum ;  rho_t = relu(0.1 p_t) [fp16]
                # both on DVE so PE's per-step wait on rho dominates the
                # psum-read WARs
                pprev = pzero if c == 0 else p_sb[(c - 1) % 2]
                pcur = p_sb[c % 2]
                for q in range(2):
                    src = psum[b][q].rearrange("p (h x) -> p h x", h=2)[:, :, s * 16 : (s + 1) * 16]
                    dst = pcur.rearrange("p (h x) -> p h x", h=4)[:, 2 * q : 2 * q + 2, :]
                    prv = pprev.rearrange("p (h x) -> p h x", h=4)[:, 2 * q : 2 * q + 2, :]
                    nc.vector.scalar_tensor_tensor(
                        out=dst, in0=prv, scalar=DECAY, in1=src, op0=AOT.mult, op1=AOT.add
                    )
                    if RELU_ENGINE == "act":
                        nc.scalar.activation(
                            out=rho[c % 2][:, q * 32 : (q + 1) * 32],
                            in_=pcur[:, q * 32 : (q + 1) * 32],
                            func=mybir.ActivationFunctionType.Relu,
                            scale=ALPHA,
                        )
                    else:
                        nc.vector.tensor_scalar(
                            rho[c % 2][:, q * 32 : (q + 1) * 32],
                            pcur[:, q * 32 : (q + 1) * 32],
                            0.0,
                            ALPHA,
                            AOT.max,
                            AOT.mult,
                        )

                # h_t = 0.9 h_{t-1} + rho_t   (fp32 history, GpSimd)
                g8, sl = divmod(c, 8)
                G = g8 % 2
                hout = hist[G].rearrange("p (m x) -> p m x", m=4)[:, :, sl * 16 : (sl + 1) * 16]
                if c == 0:
                    hprev = hzero.rearrange("p (m x) -> p m x", m=4)
                elif sl == 0:
                    hprev = hist[1 - G].rearrange("p (m x) -> p m x", m=4)[:, :, 7 * 16 : 8 * 16]
                else:
                    hprev = hist[G].rearrange("p (m x) -> p m x", m=4)[:, :, (sl - 1) * 16 : sl * 16]
                nc.vector.scalar_tensor_tensor(
                    out=hout,
                    in0=hprev,
                    scalar=DECAY,
                    in1=rho[c % 2].rearrange("p (m x) -> p m x", m=4),
                    op0=AOT.mult,
                    op1=AOT.add,
                )

                # queue the 8-step output group (transpose -> ACT copy -> DMA),
                # spread over the following steps as PE/ACT stall filler
                if sl == 7:
                    def make_group_ops(g8=g8, G=G):
                        ops = []
                        if g8 >= 1:
                            # dummy transpose absorbing the ACT copy-WAR on psumT
                            ops.append(lambda: nc.tensor.transpose(
                                psumT[0:1, 0:1], ident[0:1, 0:1], ident[0:1, 0:1]))
                        for m in range(4):
                            def tr(m=m):
                                nc.tensor.transpose(
                                    psumT[:, m * 128 : (m + 1) * 128],
                                    hist[G][:, m * 128 : (m + 1) * 128],
                                    ident[:],
                                )
                                nc.scalar.copy(
                                    out=stag[G][:, m * 128 : (m + 1) * 128],
                                    in_=psumT[:, m * 128 : (m + 1) * 128],
                                )
                            ops.append(tr)
                        ops.append(lambda: nc.sync.dma_start(
                            out=out_d[g8 * 8 : (g8 + 1) * 8].rearrange("s b h -> (s b) h"),
                            in_=stag[G][:],
                        ))
                        return ops

                    pending_out.extend(make_group_ops())

            # drain any queued output ops (last group's transposes + DMA)
            for op in pending_out:
                op()

    if split_waits:
        _split_multi_waits(nc)
    return nc


# ---------------------------------------------------------------------------
# host side
# ---------------------------------------------------------------------------

def prep_weights(Wi, bi, W_raw, bh):
    Wi = np.asarray(Wi, np.float32)
    W_raw = np.asarray(W_raw, np.float32)
    b = np.asarray(bi, np.float32) + np.asarray(bh, np.float32)

    eye = np.eye(HID, dtype=np.float32)
    W_eff = eye - W_raw.T @ W_raw - np.float32(1e-5) * eye  # [H, H], symmetric

    # lhsT tile (k, m) = W_eff[k-block, m-block]  (symmetry)
    w16 = np.empty((128, 16 * 128), np.float16)
    for k in range(4):
        for m in range(4):
            w16[:, (k * 4 + m) * 128 : (k * 4 + m + 1) * 128] = W_eff[
                k * 128 : (k + 1) * 128, m * 128 : (m + 1) * 128
            ].astype(np.float16)

    wit = np.zeros((128, 1536), np.float32)
    WiT = Wi.T  # [IN, HID]
    wit[:, 0:512] = WiT[0:128]
    wit[:, 512:1024] = WiT[128:256]
    wit[0, 1024:1536] = b
    return w16, wit


def prep_xd(x_core, n_steps=None):
    """x_core [S, bc, IN] -> packed xd [128, nchunks*768]."""
    S = x_core.shape[0] if n_steps is None else n_steps
    nchunks = S // CHUNK
    d = np.empty((S, x_core.shape[1], IN), np.float32)
    d[0] = x_core[0]
    d[1:] = x_core[1:S] - np.float32(DECAY) * x_core[: S - 1]
    dT = d.transpose(2, 0, 1)  # [IN, S, bc]
    xd = np.zeros((128, nchunks * 768), np.float32)
    for j in range(nchunks):
        blk = dT[:, j * CHUNK : (j + 1) * CHUNK, :].reshape(IN, CHUNK * BC)
        xd[:, j * 768 : j * 768 + 256] = blk[0:128]
        xd[:, j * 768 + 256 : j * 768 + 512] = blk[128:256]
        kap = np.full((CHUNK * BC,), ALPHA, np.float32)
        if j == 0:
            kap[:BC] = 1.0
        xd[0, j * 768 + 512 : (j + 1) * 768] = kap
    return xd


_CACHE = {}


def _get_nc(n_steps):
    if n_steps not in _CACHE:
        _CACHE[n_steps] = build_nc(n_steps)
    return _CACHE[n_steps]


def kernel(x, Wi, bi, W_raw, bh):
    from concourse.bass_utils import run_bass_kernel_spmd

    x = np.asarray(x, np.float32)
    w16, wit = prep_weights(Wi, bi, W_raw, bh)

    in_maps = []
    for i in range(NCORES):
        xd = prep_xd(x[:, i * BC : (i + 1) * BC, :])
        in_maps.append({"xd": xd, "w16": w16, "wit": wit})

    nc = _get_nc(SEQ)
    res = run_bass_kernel_spmd(nc, in_maps, list(range(NCORES)))
    outputs = np.concatenate([res.results[i]["out_bh"] for i in range(NCORES)], axis=1)
    h_final = outputs[-1].copy()
    return outputs, h_final
